# revision 4
# baseline (speedup 1.0000x reference)
"""AttentionPerLabelWordLevel Trainium2 kernel (8-core SPMD, batch-sharded).

Reference computation (per batch b):
  h = tanh(x @ W.T + b)                      # [T, H]
  logits = h @ C.T                           # [S, L, C]
  m = max_L(logits)                          # [S, 1, C]
  attn = softmax_C(logits - m)               # [S, L, C]
  out[s, c, :] = sum_l attn[s, l, c] * x[s, l, :]   # [S, C, H]

Shapes: B=32, T=2500 (S=100 sentences x L=25 words), H=512, C=50.
Sharding: data-parallel over batch, 4 batches per core.

Per-core strategy (all matmul operands in float16 — same mantissa
class as tf32, 1 cycle/row on the PE, fast weight loads, and legal
2-byte DMA-transposes):
  - x is DMA'd once per 16-sentence wave into "packed" fp32 SBUF
    tiles [128, 512] holding 4 sentences at partition offsets
    0/32/64/96 (25 words + 7 pad rows), then cast to f16 on GpSimd.
  - x^T for the input matmul comes from SBUF->SBUF DMA-transposes
    (xbar), not the PE; same for e^T -> attention tiles.
  - h^T, logits, e live on a padded t-axis (32 slots/sentence).
  - Softmax normalization is folded into a per-word scale of the
    attention weights.
  - The output einsum runs as f16 matmuls packed 4x along K (row
    groups) and 2x along M (col groups).
  - Loads + transposes issue on the Scalar HWDGE queue, stores on
    the Sync HWDGE queue.
"""

import numpy as np

import concourse.bacc as bacc
import concourse.bass as bass
import concourse.tile as tile
from concourse import mybir
from concourse.bass_utils import run_bass_kernel_spmd

F32 = mybir.dt.float32
F16 = mybir.dt.float16
AX = mybir.AxisListType
AF = mybir.ActivationFunctionType

N_CORES = 8
B = 32
S = 100          # sentences per batch
L = 25           # words per sentence
C = 50           # classes
H = 512          # hidden
B_LOC = B // N_CORES          # batches per core
WAVE_S = 16                   # sentences per wave (4 packed tiles)
N_WAVES = 7                   # 6 full waves + 1 final wave (4 sentences)

_CACHE = {}
LAST_RESULT = None


def build_nc():
    nc = bacc.Bacc(trn_type="TRN2", target_bir_lowering=False, debug=False)
    x_d = nc.declare_dram_parameter("input_tensor", [B_LOC, S * L, H], F32, isOutput=False)
    w_d = nc.declare_dram_parameter("W", [H, H], F32, isOutput=False)
    b_d = nc.declare_dram_parameter("b", [H], F32, isOutput=False)
    c_d = nc.declare_dram_parameter("context_vector", [C, H], F32, isOutput=False)
    o_d = nc.declare_dram_parameter("out", [B_LOC, S, C, H], F32, isOutput=True)

    load_eng = nc.scalar      # HWDGE queue for loads + transposes
    store_eng = nc.sync       # HWDGE queue for stores

    with tile.TileContext(nc) as tc:
        with tc.tile_pool(name="sb", bufs=1) as sb, \
             tc.tile_pool(name="consts", bufs=1) as consts, \
             tc.tile_pool(name="ps", bufs=1, space="PSUM") as ps:

            # ---------------- one-time setup ----------------
            b_sb = consts.tile([128, 4], F32)
            load_eng.dma_start(out=b_sb, in_=b_d.rearrange("(k p) -> p k", p=128))

            # W^T tiles: W_T[i] is [i-part 128, o 512] (f16)
            w_t = []
            for i in range(4):
                wt = consts.tile([128, 512], F16, name=f"w_t{i}")
                w_t.append(wt)
            for o in range(4):
                wn = consts.tile([128, 512], F32, name=f"w_nat{o}")
                load_eng.dma_start(out=wn, in_=w_d[o * 128:(o + 1) * 128, :])
                wh = consts.tile([128, 512], F16, name=f"w_nath{o}")
                nc.gpsimd.tensor_copy(wh, wn)
                for i in range(4):
                    load_eng.dma_start_transpose(
                        w_t[i][:, o * 128:(o + 1) * 128],
                        wh[:, i * 128:(i + 1) * 128],
                    )

            # C^T tile: [o-part 128, o_chunk 4, c 50] (f16)
            c_nat = consts.tile([C, 512], F32)
            load_eng.dma_start(out=c_nat, in_=c_d[:, :])
            c_h = consts.tile([64, 512], F16)
            nc.gpsimd.tensor_copy(c_h[:C, :], c_nat)
            c_t = consts.tile([128, 4, 64], F16)
            for o in range(4):
                load_eng.dma_start_transpose(
                    c_t[:, o, :], c_h[:, o * 128:(o + 1) * 128]
                )

            # ---------------- main loop ----------------
            for bi in range(B_LOC):
                for wv in range(N_WAVES):
                    s0 = wv * WAVE_S
                    ns = min(WAVE_S, S - s0)      # 16 or 4
                    G = ns // 4                   # packed tiles (4 or 1)
                    W_COLS = 32 * ns              # padded t-cols (512 or 128)

                    # -- load packed x tiles, cast to f16 on gpsimd --
                    xp = []
                    for g in range(G):
                        t_ = sb.tile([128, 512], F32, tag="xp", bufs=8,
                                     name=f"xp{bi}_{wv}_{g}")
                        for jj in range(4):
                            s_ = s0 + 4 * g + jj
                            load_eng.dma_start(
                                out=t_[32 * jj:32 * jj + L, :],
                                in_=x_d[bi, s_ * L:(s_ + 1) * L, :],
                            )
                        th = sb.tile([128, 512], F16, tag="xph", bufs=8,
                                     name=f"xph{bi}_{wv}_{g}")
                        nc.gpsimd.tensor_copy(th, t_)
                        xp.append(th)

                    # -- x^T via DMA-transpose (padded t axis) --
                    xt_sb = []
                    for i in range(4):            # i-chunk
                        xs = sb.tile([128, 512], F16, tag="xt_sb", bufs=8,
                                     name=f"xt_sb{bi}_{wv}_{i}")
                        for g in range(G):
                            load_eng.dma_start_transpose(
                                xs[:, 128 * g:128 * (g + 1)],
                                xp[g][:, i * 128:(i + 1) * 128],
                            )
                        xt_sb.append(xs)

                    # -- step 1: h^T[o] = tanh(W @ x^T + b), f16 --
                    h = []
                    for o in range(4):
                        ph = ps.tile([128, W_COLS], F32, tag="ph", bufs=2,
                                     name=f"ph{bi}_{wv}_{o}")
                        for i in range(4):
                            nc.tensor.matmul(
                                ph,
                                w_t[i][:, o * 128:(o + 1) * 128],
                                xt_sb[i][:, :W_COLS],
                                start=(i == 0), stop=(i == 3),
                            )
                        ht = sb.tile([128, 512], F16, tag="h", bufs=8,
                                     name=f"h{bi}_{wv}_{o}")
                        nc.scalar.activation(
                            out=ht[:, :W_COLS], in_=ph,
                            func=AF.Tanh, bias=b_sb[:, o:o + 1], scale=1.0,
                        )
                        h.append(ht)

                    # -- step 2: logits[c, t] (accumulate over o) --
                    pl = ps.tile([C, W_COLS], F32, tag="pl", bufs=2,
                                 name=f"pl{bi}_{wv}")
                    for o in range(4):
                        nc.tensor.matmul(
                            pl, c_t[:, o, :C], h[o][:, :W_COLS],
                            start=(o == 0), stop=(o == 3),
                        )

                    # -- m = max over words (strided view skips pad cols) --
                    m = sb.tile([C, WAVE_S], F32, tag="m", bufs=2,
                                name=f"m{bi}_{wv}")
                    pl_v = bass.AP(tensor=pl.tensor, offset=pl.offset,
                                   ap=[pl.ap[0], [32, ns], [1, L]])
                    nc.vector.reduce_max(out=m[:, :ns], in_=pl_v, axis=AX.X)

                    # -- e = exp(logits - m) (strided, padded layout kept) --
                    epre = sb.tile([64, 512], F16, tag="epre", bufs=2,
                                   name=f"epre{bi}_{wv}")
                    e_sb = sb.tile([64, 512], F16, tag="e", bufs=2,
                                   name=f"e{bi}_{wv}")
                    ep_v = bass.AP(tensor=epre.tensor, offset=epre.offset,
                                   ap=[[epre.ap[0][0], C], [32, ns], [1, L]])
                    e_v = bass.AP(tensor=e_sb.tensor, offset=e_sb.offset,
                                  ap=[[e_sb.ap[0][0], C], [32, ns], [1, L]])
                    m_v = bass.AP(tensor=m.tensor, offset=m.offset,
                                  ap=[m.ap[0], [1, ns], [0, L]])
                    nc.vector.tensor_sub(ep_v, pl_v, m_v)
                    nc.scalar.activation(out=e_v, in_=ep_v, func=AF.Exp)

                    # -- e^T via DMA-transpose -> packed attn tiles, normalize --
                    attn = []
                    for g in range(G):
                        at = sb.tile([128, 64], F16, tag="attn", bufs=8,
                                     name=f"attn{bi}_{wv}_{g}")
                        load_eng.dma_start_transpose(
                            at, e_sb[:, 128 * g:128 * (g + 1)]
                        )
                        z = sb.tile([128, 1], F32, tag="z", bufs=8,
                                    name=f"z{bi}_{wv}_{g}")
                        nc.vector.reduce_sum(out=z, in_=at[:, :C], axis=AX.X)
                        nc.vector.reciprocal(out=z, in_=z)
                        nc.vector.tensor_scalar_mul(at[:, :C], at[:, :C], z)
                        attn.append(at)

                    # -- step 5: out[c, o] per sentence; 4xK 2xM packed f16 --
                    n_pairs = max(1, G // 2)
                    for pi in range(n_pairs):
                        gl_count = 2 if G >= 2 else 1
                        for jj in range(4):
                            po = ps.tile([128, 512], F32, tag=f"po{jj % 2}",
                                         bufs=2, name=f"po{bi}_{wv}_{pi}_{jj}")
                            for gl in range(gl_count):
                                g = 2 * pi + gl
                                nc.tensor.matmul(
                                    po[64 * gl:64 * gl + C, :],
                                    attn[g][32 * jj:32 * jj + L, :C],
                                    xp[g][32 * jj:32 * jj + L, :],
                                    start=True, stop=True,
                                    tile_position=(32 * jj, 64 * gl),
                                )
                            osb = sb.tile([128, 512], F32, tag="osb", bufs=8,
                                          name=f"osb{bi}_{wv}_{pi}_{jj}")
                            ncols = 64 * (gl_count - 1) + C
                            if jj % 2 == 0:
                                nc.vector.tensor_copy(
                                    osb[:ncols, :], po[:ncols, :])
                            else:
                                nc.scalar.copy(
                                    osb[:ncols, :], po[:ncols, :])
                            for gl in range(gl_count):
                                g = 2 * pi + gl
                                s_ = s0 + 4 * g + jj
                                store_eng.dma_start(
                                    out=o_d[bi, s_],
                                    in_=osb[64 * gl:64 * gl + C, :],
                                )
    nc.compile()
    return nc


def kernel(**inputs):
    global LAST_RESULT
    if "nc" not in _CACHE:
        _CACHE["nc"] = build_nc()
    nc = _CACHE["nc"]

    x = np.ascontiguousarray(inputs["input_tensor"], dtype=np.float32)
    w = np.ascontiguousarray(inputs["W"], dtype=np.float32)
    bb = np.ascontiguousarray(inputs["b"], dtype=np.float32)
    cv = np.ascontiguousarray(inputs["context_vector"], dtype=np.float32)

    in_maps = [
        {
            "input_tensor": x[ci * B_LOC:(ci + 1) * B_LOC],
            "W": w,
            "b": bb,
            "context_vector": cv,
        }
        for ci in range(N_CORES)
    ]
    res = run_bass_kernel_spmd(nc, in_maps, core_ids=list(range(N_CORES)))
    LAST_RESULT = res
    out = np.empty((B, S, C, H), dtype=np.float32)
    for ci in range(N_CORES):
        out[ci * B_LOC:(ci + 1) * B_LOC] = res.results[ci]["out"]
    return out


# revision 6
# speedup vs baseline: 2.5394x; 2.5394x over previous
"""AttentionPerLabelWordLevel Trainium2 kernel (8-core SPMD, batch-sharded).

Reference computation (per batch b):
  h = tanh(x @ W.T + b)                      # [T, H]
  logits = h @ C.T                           # [S, L, C]
  m = max_L(logits)                          # [S, 1, C]
  attn = softmax_C(logits - m)               # [S, L, C]
  out[s, c, :] = sum_l attn[s, l, c] * x[s, l, :]   # [S, C, H]

Shapes: B=32, T=2500 (S=100 sentences x L=25 words), H=512, C=50.
Sharding: data-parallel over batch, 4 batches per core.

Per-core strategy (all matmul operands in float16 — 11-bit mantissa
like tf32, 1 cycle/row on the PE, fast pipelined weight loads):
  - x is DMA'd once per 16-sentence wave (one 200 KB contiguous DMA
    per 4-sentence tile, composite dest AP) into fp32 "packed" tiles
    [128, 512] holding 4 sentences at partition offsets 0/32/64/96
    (25 words + 7 pad rows), then cast to f16 on GpSimd.
  - x^T and e^T come from f16 PE transposes (1 cycle/row) with DVE
    copies back to SBUF at the f16 2x rate.
  - h^T, logits, e live on a padded t-axis (32 slots/sentence).
  - Softmax normalization is folded into a per-word scale of the
    attention weights.
  - The output einsum runs as f16 matmuls packed 4x along K (row
    groups) and 2x along M (col groups).
  - Loads and stores are split across the Sync and Scalar HWDGE
    queues for aggregate DMA bandwidth.
"""

import numpy as np

import concourse.bacc as bacc
import concourse.bass as bass
import concourse.tile as tile
from concourse import mybir
from concourse.bass_utils import run_bass_kernel_spmd
from concourse.masks import make_identity

F32 = mybir.dt.float32
F16 = mybir.dt.float16
AX = mybir.AxisListType
AF = mybir.ActivationFunctionType

N_CORES = 8
B = 32
S = 100          # sentences per batch
L = 25           # words per sentence
C = 50           # classes
H = 512          # hidden
B_LOC = B // N_CORES          # batches per core
WAVE_S = 16                   # sentences per wave (4 packed tiles)
N_WAVES = 7                   # 6 full waves + 1 final wave (4 sentences)

_CACHE = {}
LAST_RESULT = None


def build_nc():
    nc = bacc.Bacc(trn_type="TRN2", target_bir_lowering=False, debug=False)
    x_d = nc.declare_dram_parameter("input_tensor", [B_LOC, S * L, H], F32, isOutput=False)
    w_d = nc.declare_dram_parameter("W", [H, H], F32, isOutput=False)
    b_d = nc.declare_dram_parameter("b", [H], F32, isOutput=False)
    c_d = nc.declare_dram_parameter("context_vector", [C, H], F32, isOutput=False)
    o_d = nc.declare_dram_parameter("out", [B_LOC, S, C, H], F32, isOutput=True)

    q = [nc.sync, nc.scalar]     # the two HWDGE queues

    with tile.TileContext(nc) as tc:
        with tc.tile_pool(name="sb", bufs=1) as sb, \
             tc.tile_pool(name="consts", bufs=1) as consts, \
             tc.tile_pool(name="ps", bufs=1, space="PSUM") as ps:

            # ---------------- one-time setup ----------------
            ident_f = consts.tile([128, 128], F32)
            make_identity(nc, ident_f)
            ident_h = consts.tile([128, 128], F16)
            nc.vector.tensor_copy(ident_h, ident_f)

            b_sb = consts.tile([128, 4], F32)
            nc.scalar.dma_start(out=b_sb, in_=b_d.rearrange("(k p) -> p k", p=128))

            # W^T tiles: W_T[i] is [i-part 128, o 512] (f16), via xbar
            w_t = []
            for i in range(4):
                wt = consts.tile([128, 512], F16, name=f"w_t{i}")
                w_t.append(wt)
            for o in range(4):
                wn = consts.tile([128, 512], F32, name=f"w_nat{o}")
                nc.scalar.dma_start(out=wn, in_=w_d[o * 128:(o + 1) * 128, :])
                wh = consts.tile([128, 512], F16, name=f"w_nath{o}")
                nc.gpsimd.tensor_copy(wh, wn)
                for i in range(4):
                    nc.scalar.dma_start_transpose(
                        w_t[i][:, o * 128:(o + 1) * 128],
                        wh[:, i * 128:(i + 1) * 128],
                    )

            # C^T tile: [o-part 128, o_chunk 4, c 64] (f16), via xbar
            c_nat = consts.tile([C, 512], F32)
            nc.scalar.dma_start(out=c_nat, in_=c_d[:, :])
            c_h = consts.tile([64, 512], F16)
            nc.gpsimd.tensor_copy(c_h[:C, :], c_nat)
            c_t = consts.tile([128, 4, 64], F16)
            for o in range(4):
                nc.scalar.dma_start_transpose(
                    c_t[:, o, :], c_h[:, o * 128:(o + 1) * 128]
                )

            # ---------------- main loop ----------------
            for bi in range(B_LOC):
                for wv in range(N_WAVES):
                    s0 = wv * WAVE_S
                    ns = min(WAVE_S, S - s0)      # 16 or 4
                    G = ns // 4                   # packed tiles (4 or 1)
                    W_COLS = 32 * ns              # padded t-cols (512 or 128)

                    # -- load packed x tiles (one 200 KB DMA each), cast f16 --
                    xp = []
                    for g in range(G):
                        t_ = sb.tile([128, 512], F32, tag="xp", bufs=8,
                                     name=f"xp{bi}_{wv}_{g}")
                        for jj in range(4):
                            s_ = s0 + 4 * g + jj
                            q[(g + jj) % 2].dma_start(
                                out=t_[32 * jj:32 * jj + L, :],
                                in_=x_d[bi, s_ * L:(s_ + 1) * L, :],
                            )
                        th = sb.tile([128, 512], F16, tag="xph", bufs=8,
                                     name=f"xph{bi}_{wv}_{g}")
                        nc.gpsimd.tensor_copy(th, t_)
                        xp.append(th)

                    # -- x^T via f16 PE transposes (padded t axis) --
                    xt_sb = []
                    for i in range(4):            # i-chunk
                        pxt = ps.tile([128, 512], F16, tag="xt", bufs=2,
                                      name=f"pxt{bi}_{wv}_{i}")
                        for g in range(G):
                            nc.tensor.transpose(
                                pxt[:, 128 * g:128 * (g + 1)],
                                xp[g][:, i * 128:(i + 1) * 128],
                                ident_h,
                            )
                        xs = sb.tile([128, 512], F16, tag="xt_sb", bufs=8,
                                     name=f"xt_sb{bi}_{wv}_{i}")
                        nc.vector.tensor_copy(xs[:, :W_COLS], pxt[:, :W_COLS])
                        xt_sb.append(xs)

                    # -- step 1: h^T[o] = tanh(W @ x^T + b), f16 --
                    h = []
                    for o in range(4):
                        ph = ps.tile([128, W_COLS], F32, tag="ph", bufs=2,
                                     name=f"ph{bi}_{wv}_{o}")
                        for i in range(4):
                            nc.tensor.matmul(
                                ph,
                                w_t[i][:, o * 128:(o + 1) * 128],
                                xt_sb[i][:, :W_COLS],
                                start=(i == 0), stop=(i == 3),
                            )
                        ht = sb.tile([128, 512], F16, tag="h", bufs=8,
                                     name=f"h{bi}_{wv}_{o}")
                        nc.scalar.activation(
                            out=ht[:, :W_COLS], in_=ph,
                            func=AF.Tanh, bias=b_sb[:, o:o + 1], scale=1.0,
                        )
                        h.append(ht)

                    # -- step 2: logits[c, t] (accumulate over o) --
                    pl = ps.tile([C, W_COLS], F32, tag="pl", bufs=2,
                                 name=f"pl{bi}_{wv}")
                    for o in range(4):
                        nc.tensor.matmul(
                            pl, c_t[:, o, :C], h[o][:, :W_COLS],
                            start=(o == 0), stop=(o == 3),
                        )

                    # -- m = max over words (strided view skips pad cols) --
                    m = sb.tile([C, WAVE_S], F32, tag="m", bufs=2,
                                name=f"m{bi}_{wv}")
                    pl_v = bass.AP(tensor=pl.tensor, offset=pl.offset,
                                   ap=[pl.ap[0], [32, ns], [1, L]])
                    nc.vector.reduce_max(out=m[:, :ns], in_=pl_v, axis=AX.X)

                    # -- e = exp(logits - m) (strided, padded layout kept) --
                    epre = sb.tile([C, 512], F16, tag="epre", bufs=2,
                                   name=f"epre{bi}_{wv}")
                    e_sb = sb.tile([C, 512], F16, tag="e", bufs=2,
                                   name=f"e{bi}_{wv}")
                    ep_v = bass.AP(tensor=epre.tensor, offset=epre.offset,
                                   ap=[epre.ap[0], [32, ns], [1, L]])
                    e_v = bass.AP(tensor=e_sb.tensor, offset=e_sb.offset,
                                  ap=[e_sb.ap[0], [32, ns], [1, L]])
                    m_v = bass.AP(tensor=m.tensor, offset=m.offset,
                                  ap=[m.ap[0], [1, ns], [0, L]])
                    nc.vector.tensor_sub(ep_v, pl_v, m_v)
                    nc.scalar.activation(out=e_v, in_=ep_v, func=AF.Exp)

                    # -- e^T via f16 PE transpose -> packed attn, normalize --
                    attn = []
                    for g in range(G):
                        pet = ps.tile([128, 64], F16, tag="xt", bufs=2,
                                      name=f"pet{bi}_{wv}_{g}")
                        nc.tensor.transpose(
                            pet[:, :C], e_sb[:, 128 * g:128 * (g + 1)],
                            ident_h[:C, :C],
                        )
                        at = sb.tile([128, C], F16, tag="attn", bufs=8,
                                     name=f"attn{bi}_{wv}_{g}")
                        nc.vector.tensor_copy(at, pet[:, :C])
                        z = sb.tile([128, 1], F32, tag="z", bufs=8,
                                    name=f"z{bi}_{wv}_{g}")
                        nc.vector.reduce_sum(out=z, in_=at, axis=AX.X)
                        nc.vector.reciprocal(out=z, in_=z)
                        nc.vector.tensor_scalar_mul(at, at, z)
                        attn.append(at)

                    # -- step 5: out[c, o] per sentence; 4xK 2xM packed f16 --
                    n_pairs = max(1, G // 2)
                    for pi in range(n_pairs):
                        gl_count = 2 if G >= 2 else 1
                        for jj in range(4):
                            po = ps.tile([128, 512], F32, tag=f"po{jj % 2}",
                                         bufs=1, name=f"po{bi}_{wv}_{pi}_{jj}")
                            for gl in range(gl_count):
                                g = 2 * pi + gl
                                nc.tensor.matmul(
                                    po[64 * gl:64 * gl + C, :],
                                    attn[g][32 * jj:32 * jj + L, :],
                                    xp[g][32 * jj:32 * jj + L, :],
                                    start=True, stop=True,
                                    tile_position=(32 * jj, 64 * gl),
                                )
                            osb = sb.tile([128, 512], F32, tag="osb", bufs=8,
                                          name=f"osb{bi}_{wv}_{pi}_{jj}")
                            ncols = 64 * (gl_count - 1) + C
                            if jj % 2 == 0:
                                nc.vector.tensor_copy(
                                    osb[:ncols, :], po[:ncols, :])
                            else:
                                nc.scalar.copy(
                                    osb[:ncols, :], po[:ncols, :])
                            for gl in range(gl_count):
                                g = 2 * pi + gl
                                s_ = s0 + 4 * g + jj
                                q[(jj + 1) % 2].dma_start(
                                    out=o_d[bi, s_],
                                    in_=osb[64 * gl:64 * gl + C, :],
                                )
    nc.compile()
    return nc


def kernel(**inputs):
    global LAST_RESULT
    if "nc" not in _CACHE:
        _CACHE["nc"] = build_nc()
    nc = _CACHE["nc"]

    x = np.ascontiguousarray(inputs["input_tensor"], dtype=np.float32)
    w = np.ascontiguousarray(inputs["W"], dtype=np.float32)
    bb = np.ascontiguousarray(inputs["b"], dtype=np.float32)
    cv = np.ascontiguousarray(inputs["context_vector"], dtype=np.float32)

    in_maps = [
        {
            "input_tensor": x[ci * B_LOC:(ci + 1) * B_LOC],
            "W": w,
            "b": bb,
            "context_vector": cv,
        }
        for ci in range(N_CORES)
    ]
    res = run_bass_kernel_spmd(nc, in_maps, core_ids=list(range(N_CORES)))
    LAST_RESULT = res
    out = np.empty((B, S, C, H), dtype=np.float32)
    for ci in range(N_CORES):
        out[ci * B_LOC:(ci + 1) * B_LOC] = res.results[ci]["out"]
    return out


# revision 7
# speedup vs baseline: 2.9854x; 1.1756x over previous
"""AttentionPerLabelWordLevel Trainium2 kernel (8-core SPMD, batch-sharded).

Reference computation (per batch b):
  h = tanh(x @ W.T + b)                      # [T, H]
  logits = h @ C.T                           # [S, L, C]
  m = max_L(logits)                          # [S, 1, C]
  attn = softmax_C(logits - m)               # [S, L, C]
  out[s, c, :] = sum_l attn[s, l, c] * x[s, l, :]   # [S, C, H]

Shapes: B=32, T=2500 (S=100 sentences x L=25 words), H=512, C=50.
Sharding: data-parallel over batch, 4 batches per core.

Per-core strategy (x, W, C are pre-cast to float16 on the host — an
11-bit-mantissa format that runs 1 cycle/row on the PE with fast
pipelined weight loads and halves the load DMA volume):
  - x is DMA'd once per 16-sentence wave into f16 "packed" tiles
    [128, 512] holding 4 sentences at partition offsets 0/32/64/96
    (25 words + 7 pad rows each).
  - x^T and e^T come from f16 PE transposes (1 cycle/row) into
    full-bank f16 PSUM tiles, copied back with few wide DVE/ACT ops.
  - h^T, logits, e live on a padded t-axis (32 slots/sentence).
  - Softmax normalization is a batched per-word scale of the
    attention weights (single tensor_tensor op per wave).
  - The output einsum runs as f16 matmuls packed 4x along K (row
    groups) and 2x along M (col groups).
  - DMA traffic is spread over three initiators: Sync HWDGE,
    Scalar HWDGE, and GpSimd SWDGE.
"""

import numpy as np

import concourse.bacc as bacc
import concourse.bass as bass
import concourse.tile as tile
from concourse import mybir
from concourse.bass_utils import run_bass_kernel_spmd
from concourse.masks import make_identity

F32 = mybir.dt.float32
F16 = mybir.dt.float16
AX = mybir.AxisListType
AF = mybir.ActivationFunctionType

N_CORES = 8
B = 32
S = 100          # sentences per batch
L = 25           # words per sentence
C = 50           # classes
H = 512          # hidden
B_LOC = B // N_CORES          # batches per core
WAVE_S = 16                   # sentences per wave (4 packed tiles)
N_WAVES = 7                   # 6 full waves + 1 final wave (4 sentences)

_CACHE = {}
LAST_RESULT = None


def build_nc():
    nc = bacc.Bacc(trn_type="TRN2", target_bir_lowering=False, debug=False,
                   num_swdge_queues=2)
    x_d = nc.declare_dram_parameter("input_tensor", [B_LOC, S * L, H], F16, isOutput=False)
    w_d = nc.declare_dram_parameter("W", [H, H], F16, isOutput=False)
    b_d = nc.declare_dram_parameter("b", [H], F32, isOutput=False)
    c_d = nc.declare_dram_parameter("context_vector", [C, H], F16, isOutput=False)
    o_d = nc.declare_dram_parameter("out", [B_LOC, S, C, H], F32, isOutput=True)

    q_load = [nc.sync, nc.scalar]
    q_store = [nc.gpsimd, nc.scalar, nc.gpsimd, nc.sync]

    with tile.TileContext(nc) as tc:
        with tc.tile_pool(name="sb", bufs=1) as sb, \
             tc.tile_pool(name="consts", bufs=1) as consts, \
             tc.tile_pool(name="ps", bufs=1, space="PSUM") as ps:

            # ---------------- one-time setup ----------------
            ident_f = consts.tile([128, 128], F32)
            make_identity(nc, ident_f)
            ident_h = consts.tile([128, 128], F16)
            nc.vector.tensor_copy(ident_h, ident_f)

            b_sb = consts.tile([128, 4], F32)
            nc.sync.dma_start(out=b_sb, in_=b_d.rearrange("(k p) -> p k", p=128))

            # W^T tiles: W_T[i] is [i-part 128, o 512] (f16), via xbar
            w_t = []
            for i in range(4):
                wt = consts.tile([128, 512], F16, name=f"w_t{i}")
                w_t.append(wt)
            for o in range(4):
                wh = consts.tile([128, 512], F16, name=f"w_nat{o}")
                nc.sync.dma_start(out=wh, in_=w_d[o * 128:(o + 1) * 128, :])
                for i in range(4):
                    nc.scalar.dma_start_transpose(
                        w_t[i][:, o * 128:(o + 1) * 128],
                        wh[:, i * 128:(i + 1) * 128],
                    )

            # C^T tile: [o-part 128, o_chunk 4, c 64] (f16), via xbar
            c_h = consts.tile([64, 512], F16)
            nc.sync.dma_start(out=c_h[:C, :], in_=c_d[:, :])
            c_t = consts.tile([128, 4, 64], F16)
            for o in range(4):
                nc.scalar.dma_start_transpose(
                    c_t[:, o, :], c_h[:, o * 128:(o + 1) * 128]
                )

            # ---------------- main loop ----------------
            for bi in range(B_LOC):
                for wv in range(N_WAVES):
                    s0 = wv * WAVE_S
                    ns = min(WAVE_S, S - s0)      # 16 or 4
                    G = ns // 4                   # packed tiles (4 or 1)
                    W_COLS = 32 * ns              # padded t-cols (512 or 128)

                    # -- load packed f16 x tiles --
                    xp = []
                    for g in range(G):
                        t_ = sb.tile([128, 512], F16, tag="xp", bufs=8,
                                     name=f"xp{bi}_{wv}_{g}")
                        for jj in range(4):
                            s_ = s0 + 4 * g + jj
                            q_load[(g + jj) % 2].dma_start(
                                out=t_[32 * jj:32 * jj + L, :],
                                in_=x_d[bi, s_ * L:(s_ + 1) * L, :],
                            )
                        xp.append(t_)

                    # -- x^T via f16 PE transposes into 2 full psum banks --
                    xt_sb = []
                    for half in range(2):
                        pxt = ps.tile([128, 1024], F16, tag="xt", bufs=2,
                                      name=f"pxt{bi}_{wv}_{half}")
                        for il in range(2):
                            i = 2 * half + il
                            for g in range(G):
                                nc.tensor.transpose(
                                    pxt[:, 512 * il + 128 * g:
                                        512 * il + 128 * (g + 1)],
                                    xp[g][:, i * 128:(i + 1) * 128],
                                    ident_h,
                                )
                        xs = sb.tile([128, 1024], F16, tag="xt_sb", bufs=4,
                                     name=f"xt_sb{bi}_{wv}_{half}")
                        nc.vector.tensor_copy(xs, pxt)
                        xt_sb.append(xs)

                    def xt_rhs(i):
                        return xt_sb[i // 2][:, 512 * (i % 2):
                                             512 * (i % 2) + W_COLS]

                    # -- step 1: h^T[o] = tanh(W @ x^T + b), f16 --
                    h = []
                    for o in range(4):
                        ph = ps.tile([128, W_COLS], F32, tag="ph", bufs=2,
                                     name=f"ph{bi}_{wv}_{o}")
                        for i in range(4):
                            nc.tensor.matmul(
                                ph,
                                w_t[i][:, o * 128:(o + 1) * 128],
                                xt_rhs(i),
                                start=(i == 0), stop=(i == 3),
                            )
                        ht = sb.tile([128, 512], F16, tag="h", bufs=8,
                                     name=f"h{bi}_{wv}_{o}")
                        nc.scalar.activation(
                            out=ht[:, :W_COLS], in_=ph,
                            func=AF.Tanh, bias=b_sb[:, o:o + 1], scale=1.0,
                        )
                        h.append(ht)

                    # -- step 2: logits[c, t] (accumulate over o) --
                    pl = ps.tile([C, W_COLS], F32, tag="pl", bufs=1,
                                 name=f"pl{bi}_{wv}")
                    for o in range(4):
                        nc.tensor.matmul(
                            pl, c_t[:, o, :C], h[o][:, :W_COLS],
                            start=(o == 0), stop=(o == 3),
                        )

                    # -- m = max over words (strided view skips pad cols) --
                    m = sb.tile([C, WAVE_S], F32, tag="m", bufs=2,
                                name=f"m{bi}_{wv}")
                    pl_v = bass.AP(tensor=pl.tensor, offset=pl.offset,
                                   ap=[pl.ap[0], [32, ns], [1, L]])
                    nc.vector.reduce_max(out=m[:, :ns], in_=pl_v, axis=AX.X)

                    # -- e = exp(logits - m) (strided, padded layout kept) --
                    epre = sb.tile([C, 512], F16, tag="epre", bufs=2,
                                   name=f"epre{bi}_{wv}")
                    e_sb = sb.tile([C, 512], F16, tag="e", bufs=2,
                                   name=f"e{bi}_{wv}")
                    ep_v = bass.AP(tensor=epre.tensor, offset=epre.offset,
                                   ap=[epre.ap[0], [32, ns], [1, L]])
                    e_v = bass.AP(tensor=e_sb.tensor, offset=e_sb.offset,
                                  ap=[e_sb.ap[0], [32, ns], [1, L]])
                    m_v = bass.AP(tensor=m.tensor, offset=m.offset,
                                  ap=[m.ap[0], [1, ns], [0, L]])
                    nc.vector.tensor_sub(ep_v, pl_v, m_v)
                    nc.scalar.activation(out=e_v, in_=ep_v, func=AF.Exp)

                    # -- e^T via f16 PE transposes -> one merged attn tile --
                    pet = ps.tile([128, 256], F16, tag="et", bufs=1,
                                  name=f"pet{bi}_{wv}")
                    for g in range(G):
                        nc.tensor.transpose(
                            pet[:, 64 * g:64 * g + C],
                            e_sb[:, 128 * g:128 * (g + 1)],
                            ident_h[:C, :C],
                        )
                    attn = sb.tile([128, 256], F16, tag="attn", bufs=4,
                                   name=f"attn{bi}_{wv}")
                    nc.vector.tensor_copy(attn, pet)

                    # batched normalization: z[g] = 1/sum_c, attn *= z
                    att_v = bass.AP(tensor=attn.tensor, offset=attn.offset,
                                    ap=[attn.ap[0], [64, G], [1, C]])
                    z = sb.tile([128, 4], F32, tag="z", bufs=4,
                                name=f"z{bi}_{wv}")
                    nc.vector.reduce_sum(out=z[:, :G], in_=att_v, axis=AX.X)
                    nc.vector.reciprocal(out=z[:, :G], in_=z[:, :G])
                    z_v = bass.AP(tensor=z.tensor, offset=z.offset,
                                  ap=[z.ap[0], [1, G], [0, C]])
                    nc.vector.tensor_mul(att_v, att_v, z_v)

                    # -- step 5: out[c, o] per sentence; 4xK 2xM packed f16 --
                    n_pairs = max(1, G // 2)
                    for pi in range(n_pairs):
                        gl_count = 2 if G >= 2 else 1
                        for jj in range(4):
                            po = ps.tile([128, 512], F32, tag=f"po{jj % 2}",
                                         bufs=1, name=f"po{bi}_{wv}_{pi}_{jj}")
                            for gl in range(gl_count):
                                g = 2 * pi + gl
                                nc.tensor.matmul(
                                    po[64 * gl:64 * gl + C, :],
                                    attn[32 * jj:32 * jj + L,
                                         64 * g:64 * g + C],
                                    xp[g][32 * jj:32 * jj + L, :],
                                    start=True, stop=True,
                                    tile_position=(32 * jj, 64 * gl),
                                )
                            osb = sb.tile([128, 512], F32, tag="osb", bufs=8,
                                          name=f"osb{bi}_{wv}_{pi}_{jj}")
                            ncols = 64 * (gl_count - 1) + C
                            if jj % 2 == 0:
                                nc.vector.tensor_copy(
                                    osb[:ncols, :], po[:ncols, :])
                            else:
                                nc.scalar.copy(
                                    osb[:ncols, :], po[:ncols, :])
                            for gl in range(gl_count):
                                g = 2 * pi + gl
                                s_ = s0 + 4 * g + jj
                                q_store[jj].dma_start(
                                    out=o_d[bi, s_],
                                    in_=osb[64 * gl:64 * gl + C, :],
                                )
    nc.compile()
    return nc


def kernel(**inputs):
    global LAST_RESULT
    if "nc" not in _CACHE:
        _CACHE["nc"] = build_nc()
    nc = _CACHE["nc"]

    x = np.asarray(inputs["input_tensor"], dtype=np.float32).astype(np.float16)
    w = np.asarray(inputs["W"], dtype=np.float32).astype(np.float16)
    bb = np.ascontiguousarray(inputs["b"], dtype=np.float32)
    cv = np.asarray(inputs["context_vector"], dtype=np.float32).astype(np.float16)

    in_maps = [
        {
            "input_tensor": np.ascontiguousarray(x[ci * B_LOC:(ci + 1) * B_LOC]),
            "W": w,
            "b": bb,
            "context_vector": cv,
        }
        for ci in range(N_CORES)
    ]
    res = run_bass_kernel_spmd(nc, in_maps, core_ids=list(range(N_CORES)))
    LAST_RESULT = res
    out = np.empty((B, S, C, H), dtype=np.float32)
    for ci in range(N_CORES):
        out[ci * B_LOC:(ci + 1) * B_LOC] = res.results[ci]["out"]
    return out


# revision 8
# speedup vs baseline: 3.3040x; 1.1067x over previous
"""AttentionPerLabelWordLevel Trainium2 kernel (8-core SPMD, batch-sharded).

Reference computation (per batch b):
  h = tanh(x @ W.T + b)                      # [T, H]
  logits = h @ C.T                           # [S, L, C]
  m = max_L(logits)                          # [S, 1, C]
  attn = softmax_C(logits - m)               # [S, L, C]
  out[s, c, :] = sum_l attn[s, l, c] * x[s, l, :]   # [S, C, H]

Shapes: B=32, T=2500 (S=100 sentences x L=25 words), H=512, C=50.
Sharding: data-parallel over batch, 4 batches per core.

Per-core strategy (x, W, C are pre-cast to float16 on the host — an
11-bit-mantissa format that runs 1 cycle/row on the PE with fast
pipelined weight loads and halves the load DMA volume):
  - x is DMA'd once per 16-sentence wave into f16 "packed" tiles
    [128, 512] holding 4 sentences at partition offsets 0/32/64/96
    (25 words + 7 pad rows each).
  - x^T and e^T come from f16 PE transposes (1 cycle/row) into
    full-bank f16 PSUM tiles, copied back with few wide DVE/ACT ops.
  - h^T, logits, e live on a padded t-axis (32 slots/sentence).
  - Softmax normalization is a batched per-word scale of the
    attention weights (single tensor_tensor op per wave).
  - The output einsum runs as f16 matmuls packed 4x along K (row
    groups) and 2x along M (col groups).
  - DMA traffic is spread over three initiators: Sync HWDGE,
    Scalar HWDGE, and GpSimd SWDGE.
"""

import numpy as np

import concourse.bacc as bacc
import concourse.bass as bass
import concourse.tile as tile
from concourse import mybir
from concourse.bass_utils import run_bass_kernel_spmd
from concourse.masks import make_identity

F32 = mybir.dt.float32
F16 = mybir.dt.float16
AX = mybir.AxisListType
AF = mybir.ActivationFunctionType

N_CORES = 8
B = 32
S = 100          # sentences per batch
L = 25           # words per sentence
C = 50           # classes
H = 512          # hidden
B_LOC = B // N_CORES          # batches per core
WAVE_S = 16                   # sentences per wave (4 packed tiles)
N_WAVES = 7                   # 6 full waves + 1 final wave (4 sentences)

_CACHE = {}
LAST_RESULT = None


def build_nc():
    nc = bacc.Bacc(trn_type="TRN2", target_bir_lowering=False, debug=False,
                   num_swdge_queues=2)
    x_d = nc.declare_dram_parameter("input_tensor", [B_LOC, S * L, H], F16, isOutput=False)
    w_d = nc.declare_dram_parameter("W", [H, H], F16, isOutput=False)
    b_d = nc.declare_dram_parameter("b", [H], F32, isOutput=False)
    c_d = nc.declare_dram_parameter("context_vector", [C, H], F16, isOutput=False)
    o_d = nc.declare_dram_parameter("out", [B_LOC, S, C, H], F32, isOutput=True)

    q_load = [nc.sync, nc.scalar]
    q_store = [nc.gpsimd, nc.sync, nc.scalar]

    with tile.TileContext(nc) as tc:
        with tc.tile_pool(name="sb", bufs=1) as sb, \
             tc.tile_pool(name="consts", bufs=1) as consts, \
             tc.tile_pool(name="ps", bufs=1, space="PSUM") as ps:

            # ---------------- one-time setup ----------------
            ident_f = consts.tile([128, 128], F32)
            make_identity(nc, ident_f)
            ident_h = consts.tile([128, 128], F16)
            nc.vector.tensor_copy(ident_h, ident_f)

            b_sb = consts.tile([128, 4], F32)
            nc.sync.dma_start(out=b_sb, in_=b_d.rearrange("(k p) -> p k", p=128))

            # W^T tiles: W_T[i] is [i-part 128, o 512] (f16), via xbar
            w_t = []
            for i in range(4):
                wt = consts.tile([128, 512], F16, name=f"w_t{i}")
                w_t.append(wt)
            for o in range(4):
                wh = consts.tile([128, 512], F16, name=f"w_nat{o}")
                nc.sync.dma_start(out=wh, in_=w_d[o * 128:(o + 1) * 128, :])
                for i in range(4):
                    nc.scalar.dma_start_transpose(
                        w_t[i][:, o * 128:(o + 1) * 128],
                        wh[:, i * 128:(i + 1) * 128],
                    )

            # C^T tile: [o-part 128, o_chunk 4, c 64] (f16), via xbar
            c_h = consts.tile([64, 512], F16)
            nc.sync.dma_start(out=c_h[:C, :], in_=c_d[:, :])
            c_t = consts.tile([128, 4, 64], F16)
            for o in range(4):
                nc.scalar.dma_start_transpose(
                    c_t[:, o, :], c_h[:, o * 128:(o + 1) * 128]
                )

            # ---------------- main loop ----------------
            for bi in range(B_LOC):
                for wv in range(N_WAVES):
                    s0 = wv * WAVE_S
                    ns = min(WAVE_S, S - s0)      # 16 or 4
                    G = ns // 4                   # packed tiles (4 or 1)
                    W_COLS = 32 * ns              # padded t-cols (512 or 128)

                    # -- load packed f16 x: one DMA per word-row-block jj --
                    xp_all = sb.tile([128, 2088], F16, tag="xp", bufs=3,
                                     name=f"xp{bi}_{wv}")
                    for jj in range(4):
                        dvw = xp_all[32 * jj:32 * jj + L, :]
                        dst = bass.AP(tensor=xp_all.tensor, offset=dvw.offset,
                                      ap=[dvw.ap[0], [520, G], [1, 512]])
                        svw = x_d[bi, (s0 + jj) * L:(s0 + jj) * L + 1, :]
                        srcv = bass.AP(tensor=svw.tensor, offset=svw.offset,
                                       ap=[[512, L], [4 * L * 512, G], [1, 512]])
                        q_load[jj % 2].dma_start(out=dst, in_=srcv)

                    def xp(g):
                        return xp_all[:, 520 * g:520 * g + 512]

                    # -- x^T via f16 PE transposes into 2 full psum banks --
                    xt_sb = []
                    for half in range(2):
                        pxt = ps.tile([128, 1024], F16, tag="xt", bufs=1,
                                      name=f"pxt{bi}_{wv}_{half}")
                        for il in range(2):
                            i = 2 * half + il
                            for g in range(G):
                                nc.tensor.transpose(
                                    pxt[:, 512 * il + 128 * g:
                                        512 * il + 128 * (g + 1)],
                                    xp(g)[:, i * 128:(i + 1) * 128],
                                    ident_h,
                                )
                        xs = sb.tile([128, 1024], F16, tag="xt_sb", bufs=4,
                                     name=f"xt_sb{bi}_{wv}_{half}")
                        nc.vector.tensor_copy(xs, pxt)
                        xt_sb.append(xs)

                    def xt_rhs(i):
                        return xt_sb[i // 2][:, 512 * (i % 2):
                                             512 * (i % 2) + W_COLS]

                    # -- step 1: h^T[o] = tanh(W @ x^T + b), f16 --
                    h = []
                    for o in range(4):
                        ph = ps.tile([128, W_COLS], F32, tag="ph", bufs=3,
                                     name=f"ph{bi}_{wv}_{o}")
                        for i in range(4):
                            nc.tensor.matmul(
                                ph,
                                w_t[i][:, o * 128:(o + 1) * 128],
                                xt_rhs(i),
                                start=(i == 0), stop=(i == 3),
                            )
                        ht = sb.tile([128, 512], F16, tag="h", bufs=8,
                                     name=f"h{bi}_{wv}_{o}")
                        nc.scalar.activation(
                            out=ht[:, :W_COLS], in_=ph,
                            func=AF.Tanh, bias=b_sb[:, o:o + 1], scale=1.0,
                        )
                        h.append(ht)

                    # -- step 2: logits[c, t] (accumulate over o) --
                    pl = ps.tile([C, W_COLS], F32, tag="pl", bufs=1,
                                 name=f"pl{bi}_{wv}")
                    for o in range(4):
                        nc.tensor.matmul(
                            pl, c_t[:, o, :C], h[o][:, :W_COLS],
                            start=(o == 0), stop=(o == 3),
                        )

                    # -- m = max over words (strided view skips pad cols) --
                    m = sb.tile([C, WAVE_S], F32, tag="m", bufs=2,
                                name=f"m{bi}_{wv}")
                    pl_v = bass.AP(tensor=pl.tensor, offset=pl.offset,
                                   ap=[pl.ap[0], [32, ns], [1, L]])
                    nc.vector.reduce_max(out=m[:, :ns], in_=pl_v, axis=AX.X)

                    # -- e = exp(logits - m) (strided, padded layout kept) --
                    epre = sb.tile([C, 512], F16, tag="epre", bufs=2,
                                   name=f"epre{bi}_{wv}")
                    e_sb = sb.tile([C, 512], F16, tag="e", bufs=2,
                                   name=f"e{bi}_{wv}")
                    ep_v = bass.AP(tensor=epre.tensor, offset=epre.offset,
                                   ap=[epre.ap[0], [32, ns], [1, L]])
                    e_v = bass.AP(tensor=e_sb.tensor, offset=e_sb.offset,
                                  ap=[e_sb.ap[0], [32, ns], [1, L]])
                    m_v = bass.AP(tensor=m.tensor, offset=m.offset,
                                  ap=[m.ap[0], [1, ns], [0, L]])
                    nc.vector.tensor_sub(ep_v, pl_v, m_v)
                    nc.scalar.activation(out=e_v, in_=ep_v, func=AF.Exp)

                    # -- e^T via f16 PE transposes -> one merged attn tile --
                    pet = ps.tile([128, 256], F16, tag="et", bufs=1,
                                  name=f"pet{bi}_{wv}")
                    for g in range(G):
                        nc.tensor.transpose(
                            pet[:, 64 * g:64 * g + C],
                            e_sb[:, 128 * g:128 * (g + 1)],
                            ident_h[:C, :C],
                        )
                    attn = sb.tile([128, 256], F16, tag="attn", bufs=4,
                                   name=f"attn{bi}_{wv}")
                    nc.vector.tensor_copy(attn, pet)

                    # batched normalization: z[g] = 1/sum_c, attn *= z
                    att_v = bass.AP(tensor=attn.tensor, offset=attn.offset,
                                    ap=[attn.ap[0], [64, G], [1, C]])
                    z = sb.tile([128, 4], F32, tag="z", bufs=4,
                                name=f"z{bi}_{wv}")
                    nc.vector.reduce_sum(out=z[:, :G], in_=att_v, axis=AX.X)
                    nc.vector.reciprocal(out=z[:, :G], in_=z[:, :G])
                    z_v = bass.AP(tensor=z.tensor, offset=z.offset,
                                  ap=[z.ap[0], [1, G], [0, C]])
                    nc.vector.tensor_mul(att_v, att_v, z_v)

                    # -- step 5: out[c, o] per sentence; 4xK 2xM packed f16 --
                    n_pairs = max(1, G // 2)
                    si = 0
                    for pi in range(n_pairs):
                        gl_count = 2 if G >= 2 else 1
                        osb = sb.tile([128, 2088], F32, tag="osb", bufs=4,
                                      name=f"osb{bi}_{wv}_{pi}")
                        for jj in range(4):
                            po = ps.tile([128, 512], F32, tag=f"po{jj % 2}",
                                         bufs=1, name=f"po{bi}_{wv}_{pi}_{jj}")
                            for gl in range(gl_count):
                                g = 2 * pi + gl
                                nc.tensor.matmul(
                                    po[64 * gl:64 * gl + C, :],
                                    attn[32 * jj:32 * jj + L,
                                         64 * g:64 * g + C],
                                    xp(g)[32 * jj:32 * jj + L, :],
                                    start=True, stop=True,
                                    tile_position=(32 * jj, 64 * gl),
                                )
                            ncols = 64 * (gl_count - 1) + C
                            dstc = osb[:ncols, 520 * jj:520 * jj + 512]
                            if jj % 2 == 0:
                                nc.vector.tensor_copy(dstc, po[:ncols, :])
                            else:
                                nc.scalar.copy(dstc, po[:ncols, :])
                        for gl in range(gl_count):
                            sbase = s0 + 8 * pi + 4 * gl
                            ovw = osb[64 * gl:64 * gl + C, :]
                            srcv = bass.AP(tensor=osb.tensor, offset=ovw.offset,
                                           ap=[ovw.ap[0], [520, 4], [1, 512]])
                            dvw = o_d[bi, sbase:sbase + 1]
                            dst = bass.AP(tensor=dvw.tensor, offset=dvw.offset,
                                          ap=[[512, C], [C * 512, 4], [1, 512]])
                            q_store[si % 3].dma_start(out=dst, in_=srcv)
                            si += 1
    nc.compile()
    return nc


def kernel(**inputs):
    global LAST_RESULT
    if "nc" not in _CACHE:
        _CACHE["nc"] = build_nc()
    nc = _CACHE["nc"]

    x = np.asarray(inputs["input_tensor"], dtype=np.float32).astype(np.float16)
    w = np.asarray(inputs["W"], dtype=np.float32).astype(np.float16)
    bb = np.ascontiguousarray(inputs["b"], dtype=np.float32)
    cv = np.asarray(inputs["context_vector"], dtype=np.float32).astype(np.float16)

    in_maps = [
        {
            "input_tensor": np.ascontiguousarray(x[ci * B_LOC:(ci + 1) * B_LOC]),
            "W": w,
            "b": bb,
            "context_vector": cv,
        }
        for ci in range(N_CORES)
    ]
    res = run_bass_kernel_spmd(nc, in_maps, core_ids=list(range(N_CORES)))
    LAST_RESULT = res
    out = np.empty((B, S, C, H), dtype=np.float32)
    for ci in range(N_CORES):
        out[ci * B_LOC:(ci + 1) * B_LOC] = res.results[ci]["out"]
    return out


# revision 9
# speedup vs baseline: 3.4779x; 1.0526x over previous
"""AttentionPerLabelWordLevel Trainium2 kernel (8-core SPMD, batch-sharded).

Reference computation (per batch b):
  h = tanh(x @ W.T + b)                      # [T, H]
  logits = h @ C.T                           # [S, L, C]
  m = max_L(logits)                          # [S, 1, C]
  attn = softmax_C(logits - m)               # [S, L, C]
  out[s, c, :] = sum_l attn[s, l, c] * x[s, l, :]   # [S, C, H]

Shapes: B=32, T=2500 (S=100 sentences x L=25 words), H=512, C=50.
Sharding: data-parallel over batch, 4 batches per core.

Per-core strategy (x, W, C are pre-cast to float16 on the host — an
11-bit-mantissa format that runs 1 cycle/row on the PE with fast
pipelined weight loads and halves the load DMA volume):
  - x is DMA'd once per 16-sentence wave into f16 "packed" tiles
    [128, 512] holding 4 sentences at partition offsets 0/32/64/96
    (25 words + 7 pad rows each).
  - x^T and e^T come from f16 PE transposes (1 cycle/row) into
    full-bank f16 PSUM tiles, copied back with few wide DVE/ACT ops.
  - h^T, logits, e live on a padded t-axis (32 slots/sentence).
  - Softmax normalization is a batched per-word scale of the
    attention weights (single tensor_tensor op per wave).
  - The output einsum runs as f16 matmuls packed 4x along K (row
    groups) and 2x along M (col groups).
  - DMA traffic is spread over three initiators: Sync HWDGE,
    Scalar HWDGE, and GpSimd SWDGE.
"""

import numpy as np

import concourse.bacc as bacc
import concourse.bass as bass
import concourse.tile as tile
from concourse import mybir
from concourse.bass_utils import run_bass_kernel_spmd
from concourse.masks import make_identity

F32 = mybir.dt.float32
F16 = mybir.dt.float16
AX = mybir.AxisListType
AF = mybir.ActivationFunctionType

N_CORES = 8
B = 32
S = 100          # sentences per batch
L = 25           # words per sentence
C = 50           # classes
H = 512          # hidden
B_LOC = B // N_CORES          # batches per core
WAVE_S = 16                   # sentences per wave (4 packed tiles)
N_WAVES = 7                   # 6 full waves + 1 final wave (4 sentences)

_CACHE = {}
LAST_RESULT = None


def build_nc():
    nc = bacc.Bacc(trn_type="TRN2", target_bir_lowering=False, debug=False,
                   num_swdge_queues=2)
    x_d = nc.declare_dram_parameter("input_tensor", [B_LOC, S * L, H], F16, isOutput=False)
    w_d = nc.declare_dram_parameter("W", [H, H], F16, isOutput=False)
    b_d = nc.declare_dram_parameter("b", [H], F32, isOutput=False)
    c_d = nc.declare_dram_parameter("context_vector", [C, H], F16, isOutput=False)
    o_d = nc.declare_dram_parameter("out", [B_LOC, S, C, H], F32, isOutput=True)

    q_load = [nc.sync, nc.scalar]
    q_store = [nc.gpsimd, nc.sync, nc.scalar]

    with tile.TileContext(nc) as tc:
        with tc.tile_pool(name="sb", bufs=1) as sb, \
             tc.tile_pool(name="consts", bufs=1) as consts, \
             tc.tile_pool(name="ps", bufs=1, space="PSUM") as ps:

            # ---------------- one-time setup ----------------
            ident_f = consts.tile([128, 128], F32)
            make_identity(nc, ident_f)
            ident_h = consts.tile([128, 128], F16)
            nc.vector.tensor_copy(ident_h, ident_f)

            b_sb = consts.tile([128, 4], F32)
            nc.sync.dma_start(out=b_sb, in_=b_d.rearrange("(k p) -> p k", p=128))

            # W^T tiles: W_T[i] is [i-part 128, o 512] (f16), via xbar
            w_t = []
            for i in range(4):
                wt = consts.tile([128, 512], F16, name=f"w_t{i}")
                w_t.append(wt)
            for o in range(4):
                wh = consts.tile([128, 512], F16, name=f"w_nat{o}")
                nc.sync.dma_start(out=wh, in_=w_d[o * 128:(o + 1) * 128, :])
                for i in range(4):
                    nc.scalar.dma_start_transpose(
                        w_t[i][:, o * 128:(o + 1) * 128],
                        wh[:, i * 128:(i + 1) * 128],
                    )

            # C^T tile: [o-part 128, o_chunk 4, c 64] (f16), via xbar
            c_h = consts.tile([64, 512], F16)
            nc.sync.dma_start(out=c_h[:C, :], in_=c_d[:, :])
            c_t = consts.tile([128, 4, 64], F16)
            for o in range(4):
                nc.scalar.dma_start_transpose(
                    c_t[:, o, :], c_h[:, o * 128:(o + 1) * 128]
                )

            # ---------------- main loop ----------------
            for bi in range(B_LOC):
                for wv in range(N_WAVES):
                    s0 = wv * WAVE_S
                    ns = min(WAVE_S, S - s0)      # 16 or 4
                    G = ns // 4                   # packed tiles (4 or 1)
                    W_COLS = 32 * ns              # padded t-cols (512 or 128)

                    # -- load packed f16 x: one DMA per word-row-block jj --
                    xp_all = sb.tile([128, 2088], F16, tag="xp", bufs=3,
                                     name=f"xp{bi}_{wv}")
                    for jj in range(4):
                        dvw = xp_all[32 * jj:32 * jj + L, :]
                        dst = bass.AP(tensor=xp_all.tensor, offset=dvw.offset,
                                      ap=[dvw.ap[0], [520, G], [1, 512]])
                        svw = x_d[bi, (s0 + jj) * L:(s0 + jj) * L + 1, :]
                        srcv = bass.AP(tensor=svw.tensor, offset=svw.offset,
                                       ap=[[512, L], [4 * L * 512, G], [1, 512]])
                        q_load[jj % 2].dma_start(out=dst, in_=srcv)

                    def xp(g):
                        return xp_all[:, 520 * g:520 * g + 512]

                    # -- x^T via f16 PE transposes into 2 full psum banks --
                    xt_sb = []
                    for half in range(2):
                        pxt = ps.tile([128, 1024], F16, tag="xt", bufs=1,
                                      name=f"pxt{bi}_{wv}_{half}")
                        for il in range(2):
                            i = 2 * half + il
                            for g in range(G):
                                nc.tensor.transpose(
                                    pxt[:, 512 * il + 128 * g:
                                        512 * il + 128 * (g + 1)],
                                    xp(g)[:, i * 128:(i + 1) * 128],
                                    ident_h,
                                )
                        xs = sb.tile([128, 1024], F16, tag="xt_sb", bufs=4,
                                     name=f"xt_sb{bi}_{wv}_{half}")
                        nc.vector.tensor_copy(xs, pxt)
                        xt_sb.append(xs)

                    def xt_rhs(i):
                        return xt_sb[i // 2][:, 512 * (i % 2):
                                             512 * (i % 2) + W_COLS]

                    # -- step 1: h^T[o] = tanh(W @ x^T + b), f16 --
                    h = []
                    for o in range(4):
                        ph = ps.tile([128, W_COLS], F32, tag="ph", bufs=2,
                                     name=f"ph{bi}_{wv}_{o}")
                        for i in range(4):
                            nc.tensor.matmul(
                                ph,
                                w_t[i][:, o * 128:(o + 1) * 128],
                                xt_rhs(i),
                                start=(i == 0), stop=(i == 3),
                            )
                        ht = sb.tile([128, 512], F16, tag="h", bufs=8,
                                     name=f"h{bi}_{wv}_{o}")
                        nc.scalar.activation(
                            out=ht[:, :W_COLS], in_=ph,
                            func=AF.Tanh, bias=b_sb[:, o:o + 1], scale=1.0,
                        )
                        h.append(ht)

                    # -- step 2: logits[c, t] (accumulate over o) --
                    pl = ps.tile([C, W_COLS], F32, tag="pl", bufs=1,
                                 name=f"pl{bi}_{wv}")
                    for o in range(4):
                        nc.tensor.matmul(
                            pl, c_t[:, o, :C], h[o][:, :W_COLS],
                            start=(o == 0), stop=(o == 3),
                        )

                    # -- m = max over words (strided view skips pad cols) --
                    m = sb.tile([C, WAVE_S], F32, tag="m", bufs=3,
                                name=f"m{bi}_{wv}")
                    pl_v = bass.AP(tensor=pl.tensor, offset=pl.offset,
                                   ap=[pl.ap[0], [32, ns], [1, L]])
                    nc.vector.reduce_max(out=m[:, :ns], in_=pl_v, axis=AX.X)

                    # -- e = exp(logits - m) (strided, padded layout kept) --
                    epre = sb.tile([C, 512], F16, tag="epre", bufs=3,
                                   name=f"epre{bi}_{wv}")
                    e_sb = sb.tile([C, 512], F16, tag="e", bufs=3,
                                   name=f"e{bi}_{wv}")
                    ep_v = bass.AP(tensor=epre.tensor, offset=epre.offset,
                                   ap=[epre.ap[0], [32, ns], [1, L]])
                    e_v = bass.AP(tensor=e_sb.tensor, offset=e_sb.offset,
                                  ap=[e_sb.ap[0], [32, ns], [1, L]])
                    m_v = bass.AP(tensor=m.tensor, offset=m.offset,
                                  ap=[m.ap[0], [1, ns], [0, L]])
                    nc.vector.tensor_sub(ep_v, pl_v, m_v)
                    nc.scalar.activation(out=e_v, in_=ep_v, func=AF.Exp)

                    # -- e^T via f16 PE transposes -> one merged attn tile --
                    pet = ps.tile([128, 256], F16, tag="xt", bufs=1,
                                  name=f"pet{bi}_{wv}")
                    for g in range(G):
                        nc.tensor.transpose(
                            pet[:, 64 * g:64 * g + C],
                            e_sb[:, 128 * g:128 * (g + 1)],
                            ident_h[:C, :C],
                        )
                    attn = sb.tile([128, 256], F16, tag="attn", bufs=4,
                                   name=f"attn{bi}_{wv}")
                    nc.vector.tensor_copy(attn, pet)

                    # batched normalization: z[g] = 1/sum_c, attn *= z
                    att_v = bass.AP(tensor=attn.tensor, offset=attn.offset,
                                    ap=[attn.ap[0], [64, G], [1, C]])
                    z = sb.tile([128, 4], F32, tag="z", bufs=4,
                                name=f"z{bi}_{wv}")
                    nc.vector.reduce_sum(out=z[:, :G], in_=att_v, axis=AX.X)
                    nc.vector.reciprocal(out=z[:, :G], in_=z[:, :G])
                    z_v = bass.AP(tensor=z.tensor, offset=z.offset,
                                  ap=[z.ap[0], [1, G], [0, C]])
                    nc.vector.tensor_mul(att_v, att_v, z_v)

                    # -- step 5: out[c, o] per sentence; 4xK 2xM packed f16 --
                    n_pairs = max(1, G // 2)
                    si = 0
                    for pi in range(n_pairs):
                        gl_count = 2 if G >= 2 else 1
                        osb = sb.tile([128, 2088], F32, tag="osb", bufs=4,
                                      name=f"osb{bi}_{wv}_{pi}")
                        for jj in range(4):
                            po = ps.tile([128, 512], F32, tag=f"po{jj % 2}",
                                         bufs=2, name=f"po{bi}_{wv}_{pi}_{jj}")
                            for gl in range(gl_count):
                                g = 2 * pi + gl
                                nc.tensor.matmul(
                                    po[64 * gl:64 * gl + C, :],
                                    attn[32 * jj:32 * jj + L,
                                         64 * g:64 * g + C],
                                    xp(g)[32 * jj:32 * jj + L, :],
                                    start=True, stop=True,
                                    tile_position=(32 * jj, 64 * gl),
                                )
                            ncols = 64 * (gl_count - 1) + C
                            dstc = osb[:ncols, 520 * jj:520 * jj + 512]
                            if jj % 2 == 0:
                                nc.vector.tensor_copy(dstc, po[:ncols, :])
                            else:
                                nc.scalar.copy(dstc, po[:ncols, :])
                        for gl in range(gl_count):
                            sbase = s0 + 8 * pi + 4 * gl
                            ovw = osb[64 * gl:64 * gl + C, :]
                            srcv = bass.AP(tensor=osb.tensor, offset=ovw.offset,
                                           ap=[ovw.ap[0], [520, 4], [1, 512]])
                            dvw = o_d[bi, sbase:sbase + 1]
                            dst = bass.AP(tensor=dvw.tensor, offset=dvw.offset,
                                          ap=[[512, C], [C * 512, 4], [1, 512]])
                            q_store[si % 3].dma_start(out=dst, in_=srcv)
                            si += 1
    nc.compile()
    return nc


def kernel(**inputs):
    global LAST_RESULT
    if "nc" not in _CACHE:
        _CACHE["nc"] = build_nc()
    nc = _CACHE["nc"]

    x = np.asarray(inputs["input_tensor"], dtype=np.float32).astype(np.float16)
    w = np.asarray(inputs["W"], dtype=np.float32).astype(np.float16)
    bb = np.ascontiguousarray(inputs["b"], dtype=np.float32)
    cv = np.asarray(inputs["context_vector"], dtype=np.float32).astype(np.float16)

    in_maps = [
        {
            "input_tensor": np.ascontiguousarray(x[ci * B_LOC:(ci + 1) * B_LOC]),
            "W": w,
            "b": bb,
            "context_vector": cv,
        }
        for ci in range(N_CORES)
    ]
    res = run_bass_kernel_spmd(nc, in_maps, core_ids=list(range(N_CORES)))
    LAST_RESULT = res
    out = np.empty((B, S, C, H), dtype=np.float32)
    for ci in range(N_CORES):
        out[ci * B_LOC:(ci + 1) * B_LOC] = res.results[ci]["out"]
    return out


# revision 10
# speedup vs baseline: 3.5849x; 1.0308x over previous
"""AttentionPerLabelWordLevel Trainium2 kernel (8-core SPMD, batch-sharded).

Reference computation (per batch b):
  h = tanh(x @ W.T + b)                      # [T, H]
  logits = h @ C.T                           # [S, L, C]
  m = max_L(logits)                          # [S, 1, C]
  attn = softmax_C(logits - m)               # [S, L, C]
  out[s, c, :] = sum_l attn[s, l, c] * x[s, l, :]   # [S, C, H]

Shapes: B=32, T=2500 (S=100 sentences x L=25 words), H=512, C=50.
Sharding: data-parallel over batch, 4 batches per core.

Per-core strategy (x, W, C are pre-cast to float16 on the host — an
11-bit-mantissa format that runs 1 cycle/row on the PE with fast
pipelined weight loads and halves the load DMA volume):
  - x is DMA'd once per 16-sentence wave into f16 "packed" tiles
    [128, 512] holding 4 sentences at partition offsets 0/32/64/96
    (25 words + 7 pad rows each).
  - x^T and e^T come from f16 PE transposes (1 cycle/row) into
    full-bank f16 PSUM tiles, copied back with few wide DVE/ACT ops.
  - h^T, logits, e live on a padded t-axis (32 slots/sentence).
  - Softmax normalization is a batched per-word scale of the
    attention weights (single tensor_tensor op per wave).
  - The output einsum runs as f16 matmuls packed 4x along K (row
    groups) and 2x along M (col groups).
  - DMA traffic is spread over three initiators: Sync HWDGE,
    Scalar HWDGE, and GpSimd SWDGE.
"""

import numpy as np

import concourse.bacc as bacc
import concourse.bass as bass
import concourse.tile as tile
from concourse import mybir
from concourse.bass_utils import run_bass_kernel_spmd
from concourse.masks import make_identity

F32 = mybir.dt.float32
F16 = mybir.dt.float16
AX = mybir.AxisListType
AF = mybir.ActivationFunctionType

N_CORES = 8
B = 32
S = 100          # sentences per batch
L = 25           # words per sentence
C = 50           # classes
H = 512          # hidden
B_LOC = B // N_CORES          # batches per core
WAVE_S = 16                   # sentences per wave (4 packed tiles)
N_WAVES = 7                   # 6 full waves + 1 final wave (4 sentences)

_CACHE = {}
LAST_RESULT = None


def build_nc():
    nc = bacc.Bacc(trn_type="TRN2", target_bir_lowering=False, debug=False,
                   num_swdge_queues=2)
    x_d = nc.declare_dram_parameter("input_tensor", [B_LOC, S * L, H], F16, isOutput=False)
    w_d = nc.declare_dram_parameter("W", [H, H], F16, isOutput=False)
    b_d = nc.declare_dram_parameter("b", [H], F32, isOutput=False)
    c_d = nc.declare_dram_parameter("context_vector", [C, H], F16, isOutput=False)
    o_d = nc.declare_dram_parameter("out", [B_LOC, S, C, H], F32, isOutput=True)

    q_load = [nc.sync, nc.scalar]
    q_store = [nc.gpsimd, nc.sync, nc.scalar]

    with tile.TileContext(nc) as tc:
        with tc.tile_pool(name="sb", bufs=1) as sb, \
             tc.tile_pool(name="consts", bufs=1) as consts, \
             tc.tile_pool(name="ps", bufs=1, space="PSUM") as ps:

            # ---------------- one-time setup ----------------
            ident_f = consts.tile([128, 128], F32)
            make_identity(nc, ident_f)
            ident_h = consts.tile([128, 128], F16)
            nc.vector.tensor_copy(ident_h, ident_f)

            b_sb = consts.tile([128, 4], F32)
            nc.sync.dma_start(out=b_sb, in_=b_d.rearrange("(k p) -> p k", p=128))

            # W^T tiles: W_T[i] is [i-part 128, o 512] (f16), via xbar
            w_t = []
            for i in range(4):
                wt = consts.tile([128, 512], F16, name=f"w_t{i}")
                w_t.append(wt)
            for o in range(4):
                wh = consts.tile([128, 512], F16, name=f"w_nat{o}")
                nc.sync.dma_start(out=wh, in_=w_d[o * 128:(o + 1) * 128, :])
                for i in range(4):
                    nc.scalar.dma_start_transpose(
                        w_t[i][:, o * 128:(o + 1) * 128],
                        wh[:, i * 128:(i + 1) * 128],
                    )

            # C^T tile: [o-part 128, o_chunk 4, c 64] (f16), via xbar
            c_h = consts.tile([64, 512], F16)
            nc.sync.dma_start(out=c_h[:C, :], in_=c_d[:, :])
            c_t = consts.tile([128, 4, 64], F16)
            for o in range(4):
                nc.scalar.dma_start_transpose(
                    c_t[:, o, :], c_h[:, o * 128:(o + 1) * 128]
                )

            # ---------------- main loop ----------------
            for bi in range(B_LOC):
                for wv in range(N_WAVES):
                    s0 = wv * WAVE_S
                    ns = min(WAVE_S, S - s0)      # 16 or 4
                    G = ns // 4                   # packed tiles (4 or 1)
                    W_COLS = 32 * ns              # padded t-cols (512 or 128)

                    # -- load packed f16 x: one DMA per word-row-block jj --
                    xp_all = sb.tile([128, 2088], F16, tag="xp", bufs=3,
                                     name=f"xp{bi}_{wv}")
                    for jj in range(4):
                        dvw = xp_all[32 * jj:32 * jj + L, :]
                        dst = bass.AP(tensor=xp_all.tensor, offset=dvw.offset,
                                      ap=[dvw.ap[0], [520, G], [1, 512]])
                        svw = x_d[bi, (s0 + jj) * L:(s0 + jj) * L + 1, :]
                        srcv = bass.AP(tensor=svw.tensor, offset=svw.offset,
                                       ap=[[512, L], [4 * L * 512, G], [1, 512]])
                        q_load[jj % 2].dma_start(out=dst, in_=srcv)

                    def xp(g):
                        return xp_all[:, 520 * g:520 * g + 512]

                    # -- x^T via f16 PE transposes into 2 full psum banks --
                    xt_sb = []
                    for half in range(2):
                        pxt = ps.tile([128, 1024], F16, tag="xt", bufs=2,
                                      name=f"pxt{bi}_{wv}_{half}")
                        for il in range(2):
                            i = 2 * half + il
                            for g in range(G):
                                nc.tensor.transpose(
                                    pxt[:, 512 * il + 128 * g:
                                        512 * il + 128 * (g + 1)],
                                    xp(g)[:, i * 128:(i + 1) * 128],
                                    ident_h,
                                )
                        xs = sb.tile([128, 1024], F16, tag="xt_sb", bufs=4,
                                     name=f"xt_sb{bi}_{wv}_{half}")
                        nc.vector.tensor_copy(xs.bitcast(F32), pxt.bitcast(F32))
                        xt_sb.append(xs)

                    def xt_rhs(i):
                        return xt_sb[i // 2][:, 512 * (i % 2):
                                             512 * (i % 2) + W_COLS]

                    # -- step 1: h^T[o] = tanh(W @ x^T + b), f16 --
                    h = []
                    for o in range(4):
                        ph = ps.tile([128, W_COLS], F32, tag="ph", bufs=2,
                                     name=f"ph{bi}_{wv}_{o}")
                        for i in range(4):
                            nc.tensor.matmul(
                                ph,
                                w_t[i][:, o * 128:(o + 1) * 128],
                                xt_rhs(i),
                                start=(i == 0), stop=(i == 3),
                            )
                        ht = sb.tile([128, 512], F16, tag="h", bufs=8,
                                     name=f"h{bi}_{wv}_{o}")
                        nc.scalar.activation(
                            out=ht[:, :W_COLS], in_=ph,
                            func=AF.Tanh, bias=b_sb[:, o:o + 1], scale=1.0,
                        )
                        h.append(ht)

                    # -- step 2: logits[c, t] (accumulate over o) --
                    pl = ps.tile([C, W_COLS], F32, tag="po0", bufs=2,
                                 name=f"pl{bi}_{wv}")
                    for o in range(4):
                        nc.tensor.matmul(
                            pl, c_t[:, o, :C], h[o][:, :W_COLS],
                            start=(o == 0), stop=(o == 3),
                        )

                    # -- m = max over words (strided view skips pad cols) --
                    m = sb.tile([C, WAVE_S], F32, tag="m", bufs=3,
                                name=f"m{bi}_{wv}")
                    pl_v = bass.AP(tensor=pl.tensor, offset=pl.offset,
                                   ap=[pl.ap[0], [32, ns], [1, L]])
                    nc.vector.reduce_max(out=m[:, :ns], in_=pl_v, axis=AX.X)

                    # -- e = exp(logits - m) (strided, padded layout kept) --
                    epre = sb.tile([C, 512], F16, tag="epre", bufs=3,
                                   name=f"epre{bi}_{wv}")
                    e_sb = sb.tile([C, 512], F16, tag="e", bufs=3,
                                   name=f"e{bi}_{wv}")
                    ep_v = bass.AP(tensor=epre.tensor, offset=epre.offset,
                                   ap=[epre.ap[0], [32, ns], [1, L]])
                    e_v = bass.AP(tensor=e_sb.tensor, offset=e_sb.offset,
                                  ap=[e_sb.ap[0], [32, ns], [1, L]])
                    m_v = bass.AP(tensor=m.tensor, offset=m.offset,
                                  ap=[m.ap[0], [1, ns], [0, L]])
                    nc.vector.tensor_sub(ep_v, pl_v, m_v)
                    nc.scalar.activation(out=e_v, in_=ep_v, func=AF.Exp)

                    # -- e^T via f16 PE transposes -> one merged attn tile --
                    pet = ps.tile([128, 256], F16, tag="xt", bufs=2,
                                  name=f"pet{bi}_{wv}")
                    for g in range(G):
                        nc.tensor.transpose(
                            pet[:, 64 * g:64 * g + C],
                            e_sb[:, 128 * g:128 * (g + 1)],
                            ident_h[:C, :C],
                        )
                    attn = sb.tile([128, 256], F16, tag="attn", bufs=4,
                                   name=f"attn{bi}_{wv}")
                    nc.vector.tensor_copy(attn.bitcast(F32), pet.bitcast(F32))

                    # batched normalization: z[g] = 1/sum_c, attn *= z
                    att_v = bass.AP(tensor=attn.tensor, offset=attn.offset,
                                    ap=[attn.ap[0], [64, G], [1, C]])
                    z = sb.tile([128, 4], F32, tag="z", bufs=4,
                                name=f"z{bi}_{wv}")
                    nc.vector.reduce_sum(out=z[:, :G], in_=att_v, axis=AX.X)
                    nc.vector.reciprocal(out=z[:, :G], in_=z[:, :G])
                    z_v = bass.AP(tensor=z.tensor, offset=z.offset,
                                  ap=[z.ap[0], [1, G], [0, C]])
                    nc.vector.tensor_mul(att_v, att_v, z_v)

                    # -- step 5: out[c, o] per sentence; 4xK 2xM packed f16 --
                    n_pairs = max(1, G // 2)
                    si = 0
                    for pi in range(n_pairs):
                        gl_count = 2 if G >= 2 else 1
                        osb = sb.tile([128, 2088], F32, tag="osb", bufs=4,
                                      name=f"osb{bi}_{wv}_{pi}")
                        for jj in range(4):
                            po = ps.tile([128, 512], F32, tag=f"po{jj % 2}",
                                         bufs=2, name=f"po{bi}_{wv}_{pi}_{jj}")
                            for gl in range(gl_count):
                                g = 2 * pi + gl
                                nc.tensor.matmul(
                                    po[64 * gl:64 * gl + C, :],
                                    attn[32 * jj:32 * jj + L,
                                         64 * g:64 * g + C],
                                    xp(g)[32 * jj:32 * jj + L, :],
                                    start=True, stop=True,
                                    tile_position=(32 * jj, 64 * gl),
                                )
                            ncols = 64 * (gl_count - 1) + C
                            dstc = osb[:ncols, 520 * jj:520 * jj + 512]
                            if jj % 2 == 0:
                                nc.vector.tensor_copy(dstc, po[:ncols, :])
                            else:
                                nc.scalar.copy(dstc, po[:ncols, :])
                        for gl in range(gl_count):
                            sbase = s0 + 8 * pi + 4 * gl
                            ovw = osb[64 * gl:64 * gl + C, :]
                            srcv = bass.AP(tensor=osb.tensor, offset=ovw.offset,
                                           ap=[ovw.ap[0], [520, 4], [1, 512]])
                            dvw = o_d[bi, sbase:sbase + 1]
                            dst = bass.AP(tensor=dvw.tensor, offset=dvw.offset,
                                          ap=[[512, C], [C * 512, 4], [1, 512]])
                            q_store[si % 3].dma_start(out=dst, in_=srcv)
                            si += 1
    nc.compile()
    return nc


def kernel(**inputs):
    global LAST_RESULT
    if "nc" not in _CACHE:
        _CACHE["nc"] = build_nc()
    nc = _CACHE["nc"]

    x = np.asarray(inputs["input_tensor"], dtype=np.float32).astype(np.float16)
    w = np.asarray(inputs["W"], dtype=np.float32).astype(np.float16)
    bb = np.ascontiguousarray(inputs["b"], dtype=np.float32)
    cv = np.asarray(inputs["context_vector"], dtype=np.float32).astype(np.float16)

    in_maps = [
        {
            "input_tensor": np.ascontiguousarray(x[ci * B_LOC:(ci + 1) * B_LOC]),
            "W": w,
            "b": bb,
            "context_vector": cv,
        }
        for ci in range(N_CORES)
    ]
    res = run_bass_kernel_spmd(nc, in_maps, core_ids=list(range(N_CORES)))
    LAST_RESULT = res
    out = np.empty((B, S, C, H), dtype=np.float32)
    for ci in range(N_CORES):
        out[ci * B_LOC:(ci + 1) * B_LOC] = res.results[ci]["out"]
    return out
